# revision 13
# baseline (speedup 1.0000x reference)
"""Trainium2 Bass kernel for nn_ACEEmbedAVD (gnn_message_passing).

Strategy:
  Host: counting-sort edges by src node; shard by node range across 8 cores
  (6250 nodes/core) -> no collectives. Nodes are processed in 32-node blocks
  (196/core); each block's edge list is padded to Tb 128-edge tiles (Tb from
  data) so all 8 cores run one static graph. Quads of 4 blocks produce one
  128-row output slab; elementwise work is batched over chunks of up to 4
  quads to amortize the per-instruction DVE/ACT bubble.

  Device (per core), t-innermost layouts for DVE 2x modes:
    pass A (sqrt ACT table): x2=|r|^2; s=sqrt(x2/8); env=relu(1-x2/8) bf16;
      v = r/sqrt(x2+64/289) bf16 (stored)
    pass B (trig ACT table): rad_c = cos(pi*c*s)*env via round-to-nearest
      big-constant range reduction + Abs + Sin; phi (10x8 feat, bf16);
      onehot (32 cols, bf16); per block A_blk (32n, 80f) += oh_t^T @ phi_t,
      col-tiled so a quad's 4 blocks land in one (128,80) PSUM tile;
      PE-transpose -> (80,128); stage2: B = A^T W_blockdiag (80->512).

  Host post: per-core rows -> B_a (128), B_v (3x64 -> N,64,3),
  B_d (6 sym pairs x32 -> N,32,3,3 mirrored).
"""

import sys

if "/opt/trn_rl_repo" not in sys.path:
    sys.path.insert(0, "/opt/trn_rl_repo")

import heapq

import numpy as np
import ml_dtypes

N_NODES = 50000
N_CORES = 8
NPC = 6250          # nodes per core
BLK = 32            # nodes per accumulation block
NBLK = 196          # blocks per core (196*32 = 6272 node slots)
QUADS = NBLK // 4   # 49 quads -> 128 output rows each
NF = 80             # phi features: 10 m-components x 8 radial channels
OUTW = 512          # 128 (a) + 3*64 (v) + 6*32 (d, sym pairs)
CHUNK = 4           # quads per elementwise batch

RND_C = 12582912.0  # 1.5 * 2**23: (x + C) - C == round-to-nearest(x)

_BF = ml_dtypes.bfloat16

_compiled_cache = {}


def _chunks():
    out = []
    q = 0
    while q < QUADS:
        ch = min(CHUNK, QUADS - q)
        out.append((q, ch))
        q += ch
    return out


def _build(Tb):
    from concourse import bacc, tile, mybir
    from concourse.tile_rust import add_dep_helper

    AF = mybir.ActivationFunctionType
    OP = mybir.AluOpType
    F32 = mybir.dt.float32
    BF16 = mybir.dt.bfloat16

    T = 4 * Tb          # tiles per quad
    TC = CHUNK * T      # tiles per full chunk

    nc = bacc.Bacc("TRN2", target_bir_lowering=False, debug=False)

    def rc(value, dtype=F32):
        key = (dtype, value)
        if key not in nc.const_aps.aps:
            t = nc.alloc_sbuf_tensor(f"c-{dtype.name}-{value}", [128, 1], dtype)
            nc.gpsimd.memset(t.ap(), value)
            nc.const_aps.aps[key] = t.ap()

    rc(64.0 / 289.0)
    rc(float(np.pi / 2))

    r_in = nc.dram_tensor("r", [128, 3 * QUADS * T], F32, kind="ExternalInput")
    ix_in = nc.dram_tensor("ix", [128, QUADS * T], BF16, kind="ExternalInput")
    cv_in = nc.dram_tensor("cv", [1, 7, TC], F32, kind="ExternalInput")
    io_in = nc.dram_tensor("io", [1, BLK, TC], BF16, kind="ExternalInput")
    id_in = nc.dram_tensor("idm", [128, 128], BF16, kind="ExternalInput")
    w_in = nc.dram_tensor("w", [NF, OUTW], BF16, kind="ExternalInput")
    out_p = nc.dram_tensor("out", [NBLK * BLK, OUTW], BF16, kind="ExternalOutput")

    chunks = _chunks()

    with tile.TileContext(nc) as tc:
        with (
            tc.tile_pool(name="const", bufs=1) as cpool,
            tc.tile_pool(name="storeV", bufs=len(chunks)) as poolV,
            tc.tile_pool(name="storeS", bufs=len(chunks)) as poolS,
            tc.tile_pool(name="storeE", bufs=len(chunks)) as poolE,
            tc.tile_pool(name="workA", bufs=3) as wa,
            tc.tile_pool(name="workB", bufs=2) as wb,
            tc.tile_pool(name="workC", bufs=3) as wc,
            tc.tile_pool(name="outB", bufs=3) as ob,
            tc.tile_pool(name="psum1", bufs=3, space="PSUM") as ps1,
            tc.tile_pool(name="psumT", bufs=2, space="PSUM") as psT,
            tc.tile_pool(name="psum2", bufs=2, space="PSUM") as ps2,
        ):
            cvt = cpool.tile([128, 7, TC], F32)
            nc.scalar.dma_start(out=cvt[:], in_=cv_in[:].to_broadcast([128, 7, TC]))
            iot = cpool.tile([128, BLK, TC], BF16)
            nc.scalar.dma_start(out=iot[:], in_=io_in[:].to_broadcast([128, BLK, TC]))
            idm = cpool.tile([128, 128], BF16)
            nc.scalar.dma_start(out=idm[:], in_=id_in[:])
            wsb = cpool.tile([NF, OUTW], BF16)
            nc.scalar.dma_start(out=wsb[:], in_=w_in[:])

            # ---------------- pass A: sqrt-family ----------------
            stores = []
            last_passA_act = None
            for q0, ch in chunks:
                tcs = ch * T
                rt = wa.tile([128, 3, 1, tcs], F32, tag="rt")
                nc.sync.dma_start(
                    out=rt[:, :, 0, :],
                    in_=r_in[:, 3 * q0 * T : 3 * (q0 + ch) * T],
                )
                sq = wa.tile([128, 3, tcs], F32, tag="sq")
                nc.scalar.activation(sq[:], rt[:, :, 0, :], AF.Square)
                x2 = wa.tile([128, 1, tcs], F32, tag="x2")
                nc.vector.tensor_tensor(x2[:], sq[:, 0:1, :], sq[:, 1:2, :], OP.add)
                nc.vector.tensor_tensor(x2[:], x2[:], sq[:, 2:3, :], OP.add)

                st = poolS.tile([128, 1, tcs], F32, tag="s")
                nc.scalar.activation(st[:], x2[:], AF.Sqrt, scale=0.125)
                env = poolE.tile([128, 1, 1, tcs], BF16, tag="env")
                nc.scalar.activation(
                    env[:, :, 0, :], x2[:], AF.Relu, scale=-0.125, bias=1.0
                )
                u = wa.tile([128, 1, tcs], F32, tag="u")
                ua = nc.scalar.activation(u[:], x2[:], AF.Sqrt, bias=64.0 / 289.0)
                last_passA_act = ua
                qr = wa.tile([128, 1, 1, tcs], F32, tag="qr")
                nc.vector.reciprocal(qr[:, :, 0, :], u[:])

                vt = poolV.tile([128, 3, 1, tcs], BF16, tag="v")
                nc.vector.tensor_tensor(
                    vt[:], rt[:], qr[:].to_broadcast([128, 3, 1, tcs]), OP.mult
                )
                stores.append((vt, st, env))

            # ---------------- pass B: trig + matmuls ----------------
            for ci, (q0, ch) in enumerate(chunks):
                tcs = ch * T
                vt, st, env = stores[ci]
                ixt = wc.tile([128, 1, tcs], BF16, tag="ix")
                nc.sync.dma_start(
                    out=ixt[:, 0, :], in_=ix_in[:, q0 * T : (q0 + ch) * T]
                )

                h = wc.tile([128, 7, tcs], F32, tag="h")
                nc.vector.tensor_tensor(
                    h[:], st[:].to_broadcast([128, 7, tcs]), cvt[:, :, :tcs], OP.mult
                )
                rnd = wc.tile([128, 7, tcs], F32, tag="rnd")
                nc.vector.tensor_scalar(
                    rnd[:], h[:], RND_C, RND_C, OP.add, OP.subtract
                )
                z = wc.tile([128, 7, tcs], F32, tag="z")
                nc.vector.tensor_tensor(z[:], h[:], rnd[:], OP.subtract)
                ab = wc.tile([128, 7, tcs], F32, tag="ab")
                ai = nc.scalar.activation(ab[:], z[:], AF.Abs)
                add_dep_helper(
                    ai.ins, last_passA_act.ins, sync=False,
                    reason="keep trig-set ACT ops after all sqrt-set ACT ops",
                )
                radp = wc.tile([128, 1, 7, tcs], BF16, tag="radp")
                nc.scalar.activation(
                    radp[:, 0, :, :],
                    ab[:],
                    AF.Sin,
                    scale=float(-2 * np.pi),
                    bias=float(np.pi / 2),
                )
                phi = wb.tile([128, 10, 8, tcs], BF16, tag="phi")
                nc.vector.tensor_copy(phi[:, 0:1, 0:1, :], env[:])
                nc.vector.tensor_tensor(
                    phi[:, 0:1, 1:8, :], radp[:],
                    env[:].to_broadcast([128, 1, 7, tcs]), OP.mult
                )
                nc.vector.tensor_tensor(
                    phi[:, 1:4],
                    vt[:].to_broadcast([128, 3, 8, tcs]),
                    phi[:, 0:1].to_broadcast([128, 3, 8, tcs]),
                    OP.mult,
                )
                nc.vector.tensor_tensor(
                    phi[:, 4:7],
                    vt[:, 0:1].to_broadcast([128, 3, 8, tcs]),
                    phi[:, 1:4],
                    OP.mult,
                )
                nc.vector.tensor_tensor(
                    phi[:, 7:9],
                    vt[:, 1:2].to_broadcast([128, 2, 8, tcs]),
                    phi[:, 2:4],
                    OP.mult,
                )
                nc.vector.tensor_tensor(
                    phi[:, 9:10],
                    vt[:, 2:3].to_broadcast([128, 1, 8, tcs]),
                    phi[:, 3:4],
                    OP.mult,
                )

                oh = wb.tile([128, BLK, tcs], BF16, tag="oh")
                nc.vector.tensor_tensor(
                    oh[:], ixt[:].to_broadcast([128, BLK, tcs]), iot[:, :, :tcs],
                    OP.is_equal,
                )

                for qq in range(ch):
                    toff = qq * T
                    psq = ps1.tile([128, NF], F32)
                    for tt in range(Tb):
                        for qb in range(4):
                            t = toff + qb * Tb + tt
                            nc.tensor.matmul(
                                psq[qb * BLK : (qb + 1) * BLK, :],
                                oh[:, :, t],
                                phi[:, :, :, t],
                                start=(tt == 0),
                                stop=(tt == Tb - 1),
                                tile_position=(0, qb * BLK),
                            )
                    aq = ob.tile([128, NF], BF16, tag="aq")
                    nc.scalar.copy(aq[:], psq[:])

                    pst = psT.tile([NF, 128], BF16)
                    nc.tensor.transpose(pst[:], aq[:], idm[:])
                    a2 = ob.tile([NF, 128], BF16, tag="a2")
                    nc.scalar.copy(a2[:], pst[:])

                    po = ps2.tile([128, OUTW], F32)
                    nc.tensor.matmul(po[:], a2[:], wsb[:], start=True, stop=True)
                    osb = ob.tile([128, OUTW], BF16, tag="osb")
                    nc.scalar.copy(osb[:], po[:])
                    nc.sync.dma_start(
                        out=out_p[(q0 + qq) * 128 : (q0 + qq + 1) * 128, :],
                        in_=osb[:],
                    )

    nc.compile()
    return nc


def _get_compiled(Tb):
    if Tb not in _compiled_cache:
        _compiled_cache[Tb] = _build(Tb)
    return _compiled_cache[Tb]


def _assign_nodes(counts):
    """Degree-balanced LPT assignment of nodes to the 8*196 32-slot blocks.
    Returns node_slot[n] = global slot id (block*32 + position)."""
    nblk = N_CORES * NBLK
    order = np.argsort(-counts, kind="stable")
    heap = [(0, b) for b in range(nblk)]
    heapq.heapify(heap)
    slots = np.zeros(nblk, np.int32)
    node_slot = np.empty(counts.shape[0], np.int64)
    max_load = 0
    for n in order:
        load, b = heapq.heappop(heap)
        node_slot[n] = b * BLK + slots[b]
        slots[b] += 1
        load += int(counts[n])
        max_load = max(max_load, load)
        if slots[b] < BLK:
            heapq.heappush(heap, (load, b))
    return node_slot, max_load


def _preprocess(r_ij, src):
    E = src.shape[0]
    src = np.asarray(src).astype(np.int64).ravel()
    r_ij = np.ascontiguousarray(np.asarray(r_ij, dtype=np.float32))

    counts = np.bincount(src, minlength=N_NODES)
    node_slot, max_load = _assign_nodes(counts)
    Tb = max(2, int(np.ceil(max_load / 128)))
    T = 4 * Tb

    eslot = node_slot[src]
    order = np.argsort(eslot, kind="stable")
    es = eslot[order]
    r_s = r_ij[order]

    b = es // BLK           # global block 0..1567
    loc = (es % BLK).astype(np.float32)
    c = b // NBLK           # core
    bl = b - c * NBLK       # block within core

    bcounts = np.bincount(b, minlength=N_CORES * NBLK)
    starts = np.zeros(N_CORES * NBLK + 1, np.int64)
    starts[1:] = np.cumsum(bcounts)
    rank = np.arange(E, dtype=np.int64) - starts[b]

    quad = bl // 4
    tt = (bl % 4) * Tb + rank // 128
    p = rank % 128

    rtmp = np.zeros((N_CORES, 128, 3, QUADS * T), np.float32)
    ix_dev = np.full((N_CORES, 128, QUADS * T), -1.0, np.float32)
    gt = quad * T + tt
    rtmp[c, p, :, gt] = r_s
    ix_dev[c, p, gt] = loc
    # chunk-major contiguous layout for single-DMA-per-chunk loads
    r_dev = np.zeros((N_CORES, 128, 3 * QUADS * T), np.float32)
    off = 0
    q = 0
    while q < QUADS:
        ch = min(CHUNK, QUADS - q)
        tcs = ch * T
        r_dev[:, :, off : off + 3 * tcs] = rtmp[:, :, :, q * T : (q + ch) * T].reshape(
            N_CORES, 128, 3 * tcs
        )
        off += 3 * tcs
        q += ch

    # output row (in the concatenated 8*6272-row result) for each node
    nb = node_slot // BLK
    nloc = node_slot % BLK
    nc_ = nb // NBLK
    nbl = nb - nc_ * NBLK
    rows = nc_ * (NBLK * BLK) + (nbl // 4) * 128 + (nbl % 4) * BLK + nloc
    return r_dev, ix_dev.astype(_BF), Tb, rows


def _build_w(W_a, W_v, W_d):
    w = np.zeros((NF, OUTW), np.float32)
    w[0:8, 0:128] = W_a
    for t in range(3):
        w[(1 + t) * 8 : (2 + t) * 8, 128 + 64 * t : 128 + 64 * (t + 1)] = W_v
    for qi in range(6):
        w[(4 + qi) * 8 : (5 + qi) * 8, 320 + 32 * qi : 320 + 32 * (qi + 1)] = W_d
    return w.astype(_BF)


def _make_inputs(r_dev, ix_dev, Tb, W_a, W_v, W_d):
    TC = CHUNK * 4 * Tb
    cv = np.ascontiguousarray(
        np.broadcast_to((np.arange(1, 8, dtype=np.float32) * 0.5)[None, :, None], (1, 7, TC))
    )
    io = np.ascontiguousarray(
        np.broadcast_to(np.arange(BLK, dtype=np.float32)[None, :, None], (1, BLK, TC))
    ).astype(_BF)
    idm = np.eye(128, dtype=np.float32).astype(_BF)
    w = _build_w(np.asarray(W_a, np.float32), np.asarray(W_v, np.float32),
                 np.asarray(W_d, np.float32))
    return [
        dict(r=r_dev[i], ix=ix_dev[i], cv=cv, io=io, idm=idm, w=w)
        for i in range(N_CORES)
    ]


def kernel(r_ij, src, W_a, W_v, W_d, n_nodes):
    from concourse.bass_utils import run_bass_kernel_spmd

    r_dev, ix_dev, Tb, rows = _preprocess(r_ij, src)
    nc = _get_compiled(Tb)
    in_maps = _make_inputs(r_dev, ix_dev, Tb, W_a, W_v, W_d)
    res = run_bass_kernel_spmd(nc, in_maps, core_ids=list(range(N_CORES)))
    full = np.concatenate(
        [np.asarray(res.results[i]["out"]) for i in range(N_CORES)], axis=0
    ).astype(np.float32)[rows]

    N = N_NODES
    B_a = np.ascontiguousarray(full[:, :128])
    B_v = np.ascontiguousarray(
        full[:, 128:320].reshape(N, 3, 64).transpose(0, 2, 1)
    )
    B_d6 = full[:, 320:512].reshape(N, 6, 32)
    pmap = np.array([[0, 1, 2], [1, 3, 4], [2, 4, 5]])
    B_d = np.ascontiguousarray(B_d6[:, pmap, :].transpose(0, 3, 1, 2))
    return B_a, B_v, B_d


# revision 14
# speedup vs baseline: 1.1529x; 1.1529x over previous
"""Trainium2 Bass kernel for nn_ACEEmbedAVD (gnn_message_passing).

Strategy:
  Host: counting-sort edges by src node; shard by node range across 8 cores
  (6250 nodes/core) -> no collectives. Nodes are processed in 32-node blocks
  (196/core); each block's edge list is padded to Tb 128-edge tiles (Tb from
  data) so all 8 cores run one static graph. Quads of 4 blocks produce one
  128-row output slab; elementwise work is batched over chunks of up to 4
  quads to amortize the per-instruction DVE/ACT bubble.

  Device (per core), t-innermost layouts for DVE 2x modes:
    pass A (sqrt ACT table): x2=|r|^2; s=sqrt(x2/8); env=relu(1-x2/8) bf16;
      v = r/sqrt(x2+64/289) bf16 (stored)
    pass B (trig ACT table): rad_c = cos(pi*c*s)*env via round-to-nearest
      big-constant range reduction + Abs + Sin; phi (10x8 feat, bf16);
      onehot (32 cols, bf16); per block A_blk (32n, 80f) += oh_t^T @ phi_t,
      col-tiled so a quad's 4 blocks land in one (128,80) PSUM tile;
      PE-transpose -> (80,128); stage2: B = A^T W_blockdiag (80->512).

  Host post: per-core rows -> B_a (128), B_v (3x64 -> N,64,3),
  B_d (6 sym pairs x32 -> N,32,3,3 mirrored).
"""

import sys

if "/opt/trn_rl_repo" not in sys.path:
    sys.path.insert(0, "/opt/trn_rl_repo")

import heapq

import numpy as np
import ml_dtypes

N_NODES = 50000
N_CORES = 8
NPC = 6250          # nodes per core
BLK = 32            # nodes per accumulation block
NBLK = 196          # blocks per core (196*32 = 6272 node slots)
QUADS = NBLK // 4   # 49 quads -> 128 output rows each
NF = 80             # phi features: 10 m-components x 8 radial channels
OUTW = 512          # 128 (a) + 3*64 (v) + 6*32 (d, sym pairs)
CHUNK = 4           # quads per elementwise batch

RND_C = 12582912.0  # 1.5 * 2**23: (x + C) - C == round-to-nearest(x)

_BF = ml_dtypes.bfloat16

_compiled_cache = {}


def _chunks():
    out = []
    q = 0
    while q < QUADS:
        ch = min(CHUNK, QUADS - q)
        out.append((q, ch))
        q += ch
    return out


def _build(Tb):
    from concourse import bacc, tile, mybir
    from concourse.tile_rust import add_dep_helper

    AF = mybir.ActivationFunctionType
    OP = mybir.AluOpType
    F32 = mybir.dt.float32
    BF16 = mybir.dt.bfloat16

    T = 4 * Tb          # tiles per quad
    TC = CHUNK * T      # tiles per full chunk

    nc = bacc.Bacc("TRN2", target_bir_lowering=False, debug=False)

    def rc(value, dtype=F32):
        key = (dtype, value)
        if key not in nc.const_aps.aps:
            t = nc.alloc_sbuf_tensor(f"c-{dtype.name}-{value}", [128, 1], dtype)
            nc.gpsimd.memset(t.ap(), value)
            nc.const_aps.aps[key] = t.ap()

    rc(64.0 / 289.0)
    rc(float(np.pi / 2))

    r_in = nc.dram_tensor("r", [128, 3 * QUADS * T], F32, kind="ExternalInput")
    ix_in = nc.dram_tensor("ix", [128, QUADS * T], BF16, kind="ExternalInput")
    cv_in = nc.dram_tensor("cv", [1, 7, TC], F32, kind="ExternalInput")
    io_in = nc.dram_tensor("io", [1, BLK, TC], BF16, kind="ExternalInput")
    id_in = nc.dram_tensor("idm", [128, 128], BF16, kind="ExternalInput")
    w_in = nc.dram_tensor("w", [NF, OUTW], BF16, kind="ExternalInput")
    out_p = nc.dram_tensor("out", [NBLK * BLK, OUTW], BF16, kind="ExternalOutput")

    chunks = _chunks()

    with tile.TileContext(nc) as tc:
        with (
            tc.tile_pool(name="const", bufs=1) as cpool,
            tc.tile_pool(name="storeV", bufs=len(chunks)) as poolV,
            tc.tile_pool(name="storeS", bufs=len(chunks)) as poolS,
            tc.tile_pool(name="storeE", bufs=len(chunks)) as poolE,
            tc.tile_pool(name="workA", bufs=3) as wa,
            tc.tile_pool(name="workB", bufs=2) as wb,
            tc.tile_pool(name="workC", bufs=3) as wc,
            tc.tile_pool(name="outB", bufs=3) as ob,
            tc.tile_pool(name="psum1", bufs=3, space="PSUM") as ps1,
            tc.tile_pool(name="psumT", bufs=2, space="PSUM") as psT,
            tc.tile_pool(name="psum2", bufs=2, space="PSUM") as ps2,
        ):
            cvt = cpool.tile([128, 7, TC], F32)
            nc.scalar.dma_start(out=cvt[:], in_=cv_in[:].to_broadcast([128, 7, TC]))
            iot = cpool.tile([128, BLK, TC], BF16)
            nc.scalar.dma_start(out=iot[:], in_=io_in[:].to_broadcast([128, BLK, TC]))
            idm = cpool.tile([128, 128], BF16)
            nc.scalar.dma_start(out=idm[:], in_=id_in[:])
            wsb = cpool.tile([NF, OUTW], BF16)
            nc.scalar.dma_start(out=wsb[:], in_=w_in[:])

            # ---------------- pass A: sqrt-family ----------------
            stores = []
            last_passA_act = None
            for q0, ch in chunks:
                tcs = ch * T
                rt = wa.tile([128, 3, 1, tcs], F32, tag="rt")
                for qq in range(ch):
                    nc.sync.dma_start(
                        out=rt[:, :, 0, qq * T : (qq + 1) * T],
                        in_=r_in[:, 3 * (q0 + qq) * T : 3 * (q0 + qq + 1) * T].rearrange(
                            "p (c t) -> p c t", c=3
                        ),
                    )
                sq = wa.tile([128, 3, tcs], F32, tag="sq")
                nc.scalar.activation(sq[:], rt[:, :, 0, :], AF.Square)
                x2 = wa.tile([128, 1, tcs], F32, tag="x2")
                nc.vector.tensor_tensor(x2[:], sq[:, 0:1, :], sq[:, 1:2, :], OP.add)
                nc.vector.tensor_tensor(x2[:], x2[:], sq[:, 2:3, :], OP.add)

                st = poolS.tile([128, 1, tcs], F32, tag="s")
                nc.scalar.activation(st[:], x2[:], AF.Sqrt, scale=0.125)
                env = poolE.tile([128, 1, 1, tcs], BF16, tag="env")
                nc.scalar.activation(
                    env[:, :, 0, :], x2[:], AF.Relu, scale=-0.125, bias=1.0
                )
                u = wa.tile([128, 1, tcs], F32, tag="u")
                ua = nc.scalar.activation(u[:], x2[:], AF.Sqrt, bias=64.0 / 289.0)
                last_passA_act = ua
                qr = wa.tile([128, 1, 1, tcs], F32, tag="qr")
                nc.vector.reciprocal(qr[:, :, 0, :], u[:])

                vt = poolV.tile([128, 3, 1, tcs], BF16, tag="v")
                nc.vector.tensor_tensor(
                    vt[:], rt[:], qr[:].to_broadcast([128, 3, 1, tcs]), OP.mult
                )
                stores.append((vt, st, env))

            # ---------------- pass B: trig + matmuls ----------------
            for ci, (q0, ch) in enumerate(chunks):
                tcs = ch * T
                vt, st, env = stores[ci]
                ixt = wc.tile([128, 1, tcs], BF16, tag="ix")
                for qq in range(ch):
                    nc.sync.dma_start(
                        out=ixt[:, 0, qq * T : (qq + 1) * T],
                        in_=ix_in[:, (q0 + qq) * T : (q0 + qq + 1) * T],
                    )

                h = wc.tile([128, 7, tcs], F32, tag="h")
                nc.vector.tensor_tensor(
                    h[:], st[:].to_broadcast([128, 7, tcs]), cvt[:, :, :tcs], OP.mult
                )
                rnd = wc.tile([128, 7, tcs], F32, tag="rnd")
                nc.vector.tensor_scalar(
                    rnd[:], h[:], RND_C, RND_C, OP.add, OP.subtract
                )
                z = wc.tile([128, 7, tcs], F32, tag="z")
                nc.vector.tensor_tensor(z[:], h[:], rnd[:], OP.subtract)
                ab = wc.tile([128, 7, tcs], F32, tag="ab")
                ai = nc.scalar.activation(ab[:], z[:], AF.Abs)
                add_dep_helper(
                    ai.ins, last_passA_act.ins, sync=False,
                    reason="keep trig-set ACT ops after all sqrt-set ACT ops",
                )
                radp = wc.tile([128, 1, 7, tcs], BF16, tag="radp")
                nc.scalar.activation(
                    radp[:, 0, :, :],
                    ab[:],
                    AF.Sin,
                    scale=float(-2 * np.pi),
                    bias=float(np.pi / 2),
                )
                phi = wb.tile([128, 10, 8, tcs], BF16, tag="phi")
                nc.vector.tensor_copy(phi[:, 0:1, 0:1, :], env[:])
                nc.vector.tensor_tensor(
                    phi[:, 0:1, 1:8, :], radp[:],
                    env[:].to_broadcast([128, 1, 7, tcs]), OP.mult
                )
                nc.vector.tensor_tensor(
                    phi[:, 1:4],
                    vt[:].to_broadcast([128, 3, 8, tcs]),
                    phi[:, 0:1].to_broadcast([128, 3, 8, tcs]),
                    OP.mult,
                )
                nc.vector.tensor_tensor(
                    phi[:, 4:7],
                    vt[:, 0:1].to_broadcast([128, 3, 8, tcs]),
                    phi[:, 1:4],
                    OP.mult,
                )
                nc.vector.tensor_tensor(
                    phi[:, 7:9],
                    vt[:, 1:2].to_broadcast([128, 2, 8, tcs]),
                    phi[:, 2:4],
                    OP.mult,
                )
                nc.vector.tensor_tensor(
                    phi[:, 9:10],
                    vt[:, 2:3].to_broadcast([128, 1, 8, tcs]),
                    phi[:, 3:4],
                    OP.mult,
                )

                oh = wb.tile([128, BLK, tcs], BF16, tag="oh")
                nc.vector.tensor_tensor(
                    oh[:], ixt[:].to_broadcast([128, BLK, tcs]), iot[:, :, :tcs],
                    OP.is_equal,
                )

                for qq in range(ch):
                    toff = qq * T
                    psq = ps1.tile([128, NF], F32)
                    for tt in range(Tb):
                        for qb in range(4):
                            t = toff + qb * Tb + tt
                            nc.tensor.matmul(
                                psq[qb * BLK : (qb + 1) * BLK, :],
                                oh[:, :, t],
                                phi[:, :, :, t],
                                start=(tt == 0),
                                stop=(tt == Tb - 1),
                                tile_position=(0, qb * BLK),
                            )
                    aq = ob.tile([128, NF], BF16, tag="aq")
                    nc.scalar.copy(aq[:], psq[:])

                    pst = psT.tile([NF, 128], BF16)
                    nc.tensor.transpose(pst[:], aq[:], idm[:])
                    a2 = ob.tile([NF, 128], BF16, tag="a2")
                    nc.scalar.copy(a2[:], pst[:])

                    po = ps2.tile([128, OUTW], F32)
                    nc.tensor.matmul(po[:], a2[:], wsb[:], start=True, stop=True)
                    osb = ob.tile([128, OUTW], BF16, tag="osb")
                    nc.scalar.copy(osb[:], po[:])
                    nc.sync.dma_start(
                        out=out_p[(q0 + qq) * 128 : (q0 + qq + 1) * 128, :],
                        in_=osb[:],
                    )

    nc.compile()
    return nc


def _get_compiled(Tb):
    if Tb not in _compiled_cache:
        _compiled_cache[Tb] = _build(Tb)
    return _compiled_cache[Tb]


def _assign_nodes(counts):
    """Degree-balanced LPT assignment of nodes to the 8*196 32-slot blocks.
    Returns node_slot[n] = global slot id (block*32 + position)."""
    nblk = N_CORES * NBLK
    order = np.argsort(-counts, kind="stable")
    heap = [(0, b) for b in range(nblk)]
    heapq.heapify(heap)
    slots = np.zeros(nblk, np.int32)
    node_slot = np.empty(counts.shape[0], np.int64)
    max_load = 0
    for n in order:
        load, b = heapq.heappop(heap)
        node_slot[n] = b * BLK + slots[b]
        slots[b] += 1
        load += int(counts[n])
        max_load = max(max_load, load)
        if slots[b] < BLK:
            heapq.heappush(heap, (load, b))
    return node_slot, max_load


def _preprocess(r_ij, src):
    E = src.shape[0]
    src = np.asarray(src).astype(np.int64).ravel()
    r_ij = np.ascontiguousarray(np.asarray(r_ij, dtype=np.float32))

    counts = np.bincount(src, minlength=N_NODES)
    node_slot, max_load = _assign_nodes(counts)
    Tb = max(2, int(np.ceil(max_load / 128)))
    T = 4 * Tb

    eslot = node_slot[src]
    order = np.argsort(eslot, kind="stable")
    es = eslot[order]
    r_s = r_ij[order]

    b = es // BLK           # global block 0..1567
    loc = (es % BLK).astype(np.float32)
    c = b // NBLK           # core
    bl = b - c * NBLK       # block within core

    bcounts = np.bincount(b, minlength=N_CORES * NBLK)
    starts = np.zeros(N_CORES * NBLK + 1, np.int64)
    starts[1:] = np.cumsum(bcounts)
    rank = np.arange(E, dtype=np.int64) - starts[b]

    quad = bl // 4
    tt = (bl % 4) * Tb + rank // 128
    p = rank % 128

    rtmp = np.zeros((N_CORES, 128, 3, QUADS * T), np.float32)
    ix_dev = np.full((N_CORES, 128, QUADS * T), -1.0, np.float32)
    gt = quad * T + tt
    rtmp[c, p, :, gt] = r_s
    ix_dev[c, p, gt] = loc
    # quad-major contiguous layout: per-quad (3, T) blocks
    r_dev = np.ascontiguousarray(
        rtmp.reshape(N_CORES, 128, 3, QUADS, T)
        .transpose(0, 1, 3, 2, 4)
        .reshape(N_CORES, 128, 3 * QUADS * T)
    )

    # output row (in the concatenated 8*6272-row result) for each node
    nb = node_slot // BLK
    nloc = node_slot % BLK
    nc_ = nb // NBLK
    nbl = nb - nc_ * NBLK
    rows = nc_ * (NBLK * BLK) + (nbl // 4) * 128 + (nbl % 4) * BLK + nloc
    return r_dev, ix_dev.astype(_BF), Tb, rows


def _build_w(W_a, W_v, W_d):
    w = np.zeros((NF, OUTW), np.float32)
    w[0:8, 0:128] = W_a
    for t in range(3):
        w[(1 + t) * 8 : (2 + t) * 8, 128 + 64 * t : 128 + 64 * (t + 1)] = W_v
    for qi in range(6):
        w[(4 + qi) * 8 : (5 + qi) * 8, 320 + 32 * qi : 320 + 32 * (qi + 1)] = W_d
    return w.astype(_BF)


def _make_inputs(r_dev, ix_dev, Tb, W_a, W_v, W_d):
    TC = CHUNK * 4 * Tb
    cv = np.ascontiguousarray(
        np.broadcast_to((np.arange(1, 8, dtype=np.float32) * 0.5)[None, :, None], (1, 7, TC))
    )
    io = np.ascontiguousarray(
        np.broadcast_to(np.arange(BLK, dtype=np.float32)[None, :, None], (1, BLK, TC))
    ).astype(_BF)
    idm = np.eye(128, dtype=np.float32).astype(_BF)
    w = _build_w(np.asarray(W_a, np.float32), np.asarray(W_v, np.float32),
                 np.asarray(W_d, np.float32))
    return [
        dict(r=r_dev[i], ix=ix_dev[i], cv=cv, io=io, idm=idm, w=w)
        for i in range(N_CORES)
    ]


def kernel(r_ij, src, W_a, W_v, W_d, n_nodes):
    from concourse.bass_utils import run_bass_kernel_spmd

    r_dev, ix_dev, Tb, rows = _preprocess(r_ij, src)
    nc = _get_compiled(Tb)
    in_maps = _make_inputs(r_dev, ix_dev, Tb, W_a, W_v, W_d)
    res = run_bass_kernel_spmd(nc, in_maps, core_ids=list(range(N_CORES)))
    full = np.concatenate(
        [np.asarray(res.results[i]["out"]) for i in range(N_CORES)], axis=0
    ).astype(np.float32)[rows]

    N = N_NODES
    B_a = np.ascontiguousarray(full[:, :128])
    B_v = np.ascontiguousarray(
        full[:, 128:320].reshape(N, 3, 64).transpose(0, 2, 1)
    )
    B_d6 = full[:, 320:512].reshape(N, 6, 32)
    pmap = np.array([[0, 1, 2], [1, 3, 4], [2, 4, 5]])
    B_d = np.ascontiguousarray(B_d6[:, pmap, :].transpose(0, 3, 1, 2))
    return B_a, B_v, B_d
